# revision 11
# baseline (speedup 1.0000x reference)
"""AttentionConv kernel for Trainium2 (8 NeuronCores, SPMD data-parallel over batch).

Problem: per-channel windowed softmax attention.
  q = wq @ x; k = wk @ pad(x, 3); v = wv @ pad(x, 3)       (1x1 convs = GEMMs)
  s_j[c,w] = q[c,w] * k[c,w+j],  j = 0..6
  out[c,w] = sum_j softmax_j(s)[c,w,j] * v[c,w+j]

Sharding: batch B=8 -> one batch element per core; weights replicated.
Since pad commutes with the channel-mixing GEMM, k/v are computed on the
unpadded x and written into SBUF buffers with 3 zero columns on each side.

Per-core engine mapping (v2 — window sums moved to the tensor engine):
  TensorE: 3 GEMMs (256x256 @ 256x4096) into fp32 PSUM, plus the softmax
           numerator/denominator j-sums as 7-deep accumulating matmuls with
           an identity stationary (den = sum_j e_j, num = sum_j e_j*v_j),
           which frees ~45% of the vector engine.
  ScalarE: PSUM->SBUF evacuation casts and exp.
  VectorE: windowed score mult, e*v mult, 1/den via the fast custom-DVE
           reciprocal (fp32), final out = num * rden.
The work is software-pipelined over 16 sub-chunks of 512 columns so DVE
(~5.2us), ACT (~4.8us) and PE (~4.4us) overlap per round.
"""

import sys

sys.path.insert(0, "/opt/trn_rl_repo")

import numpy as np

B, C, W = 8, 256, 4096
K7, PAD = 7, 3
SUB = 512  # attention pipeline sub-chunk (den/num PSUM bank)
GRP = 1024  # gemm group columns (one psum tile, 2 banks)
NSUB = W // SUB  # 8 per co block
NGRP = W // GRP  # 4 per co block
WP = W + 2 * PAD

_STATE = {}


def _build_nc():
    import concourse.bass as bass
    import concourse.tile as tile
    from concourse import bacc, mybir

    bf16 = mybir.dt.bfloat16
    f32 = mybir.dt.float32
    AF = mybir.ActivationFunctionType

    nc = bacc.Bacc("TRN2", target_bir_lowering=False, debug=False, num_devices=8)

    x_d = nc.declare_dram_parameter("x", [C, W], bf16, isOutput=False)
    w_d = {
        t: nc.declare_dram_parameter(f"wt{t}", [C, C], bf16, isOutput=False)
        for t in "qkv"
    }
    id_d = nc.declare_dram_parameter("ident", [128, 128], bf16, isOutput=False)
    out_d = nc.declare_dram_parameter("out", [C, W], bf16, isOutput=True)

    with tile.TileContext(nc) as tc:
        from contextlib import ExitStack

        with ExitStack() as ctx:
            persist = ctx.enter_context(tc.tile_pool(name="persist", bufs=1))
            gpsum = ctx.enter_context(tc.tile_pool(name="gpsum", bufs=2, space="PSUM"))
            dpsum = ctx.enter_context(tc.tile_pool(name="dpsum", bufs=2, space="PSUM"))
            npsum = ctx.enter_context(tc.tile_pool(name="npsum", bufs=2, space="PSUM"))
            spool = ctx.enter_context(tc.tile_pool(name="spool", bufs=3))
            epool = ctx.enter_context(tc.tile_pool(name="epool", bufs=3))
            rpool = ctx.enter_context(tc.tile_pool(name="rpool", bufs=3))
            opool = ctx.enter_context(tc.tile_pool(name="opool", bufs=3))

            # ---- persistent SBUF tensors ----
            xb = persist.tile([128, 2, W], bf16, tag="xb")
            wsb = {
                t: persist.tile([128, 2, C], bf16, name=f"wsb_{t}", tag=f"wsb_{t}")
                for t in "qkv"
            }  # w.T, ci-major blocks
            qsb = persist.tile([128, 2, W], bf16, tag="qsb")
            ksb = persist.tile([128, 2, WP], bf16, tag="ksb")
            vsb = persist.tile([128, 2, WP], bf16, tag="vsb")
            idb = persist.tile([128, 128], bf16, tag="idb")
            warm = persist.tile([128, 1], bf16, tag="warm")

            # ---- loads. Critical path (Sync queue): ident -> wq -> x piece 0,
            # which gates the first gemm. The rest (wk/wv, x pieces 1-3) is
            # triggered from the Scalar queue, idle until the first evac, so
            # the Sync queue reaches the x0 trigger ~4us sooner. ----
            nc.sync.dma_start(out=idb[:, :], in_=id_d[:, :])
            for cb in range(2):
                nc.sync.dma_start(
                    out=wsb["q"][:, cb, :], in_=w_d["q"][cb * 128 : (cb + 1) * 128, :]
                )
            for cb in range(2):
                nc.sync.dma_start(
                    out=xb[:, cb, 0 : 2 * GRP],
                    in_=x_d[cb * 128 : (cb + 1) * 128, 0 : 2 * GRP],
                )

            # zero the ACT warmup tile before the exp below reads it
            nc.gpsimd.memset(warm[:, :], 0.0)

            # First ACT instruction is a tiny exp so the one table set
            # (exp + copy) loads at t~7us, off the critical path.
            nc.scalar.activation(warm[:, :], warm[:, :], AF.Exp)

            for t in "kv":
                for cb in range(2):
                    nc.scalar.dma_start(
                        out=wsb[t][:, cb, :], in_=w_d[t][cb * 128 : (cb + 1) * 128, :]
                    )
            for cb in range(2):
                nc.scalar.dma_start(
                    out=xb[:, cb, 2 * GRP : 4 * GRP],
                    in_=x_d[cb * 128 : (cb + 1) * 128, 2 * GRP : 4 * GRP],
                )

            # zero the pad columns of k/v (gpsimd keeps these off the busy
            # engines)
            for buf in (ksb, vsb):
                for cb in range(2):
                    nc.gpsimd.memset(buf[:, cb, 0:PAD], 0.0)
                    nc.gpsimd.memset(buf[:, cb, W + PAD : WP], 0.0)

            # PE warmup on the identity tiles: the HAM clock state follows
            # sustained matmul activity, and the steady-state matmul rate for
            # the whole kernel depends on it (measured 216 vs 259 ns/matmul).
            # Split the burst so the first gemm group is not delayed: 8
            # matmuls start the ramp while the x DMA lands; 16 more are
            # emitted right after the first group (see below) to finish it.
            wps = gpsum.tile([128, 128], f32, name="wps", tag="g")

            def pe_warmup(n):
                for i in range(n):
                    nc.tensor.matmul(
                        wps[:, :],
                        idb[:, :],
                        idb[:, :],
                        start=True,
                        stop=True,
                        skip_group_check=True,
                    )

            pe_warmup(16)

            emitted = [0, 0]  # gemm groups emitted per co block

            def gemm_group(co, g):
                """q/k/v GEMM for cols [g*GRP, (g+1)*GRP) of co block."""
                co_sl = slice(co * 128, (co + 1) * 128)
                c0 = g * GRP
                for t in "qkv":
                    ps = gpsum.tile([128, GRP], f32, name=f"g{co}{g}{t}", tag="g")
                    for h in range(GRP // 512):
                        for ci in range(2):
                            nc.tensor.matmul(
                                ps[:, h * 512 : (h + 1) * 512],
                                wsb[t][:, ci, co_sl],
                                xb[:, ci, c0 + h * 512 : c0 + (h + 1) * 512],
                                start=(ci == 0),
                                stop=(ci == 1),
                            )
                    if t == "q":
                        dst = qsb[:, co, c0 : c0 + GRP]
                    else:
                        buf = ksb if t == "k" else vsb
                        dst = buf[:, co, PAD + c0 : PAD + c0 + GRP]
                    nc.scalar.copy(out=dst, in_=ps[:, :])

            def need_gemms(co, upto):
                for g in range(emitted[co], min(upto, NGRP - 1) + 1):
                    gemm_group(co, g)
                emitted[co] = max(emitted[co], min(upto, NGRP - 1) + 1)

            def sub_scores(k):
                """scores + exp for flat sub-chunk k; returns the e tile."""
                co, ch = k // NSUB, k % NSUB
                w0 = ch * SUB
                s = spool.tile([128, K7, SUB], bf16, name=f"s{k}", tag="s")
                qsl = qsb[:, co, w0 : w0 + SUB]
                ksl = ksb[:, co, w0 : w0 + SUB]
                q_bc = bass.AP(
                    tensor=qsl.tensor,
                    offset=qsl.offset,
                    ap=[qsl.ap[0], [0, K7], [1, SUB]],
                )
                k_wn = bass.AP(
                    tensor=ksl.tensor,
                    offset=ksl.offset,
                    ap=[ksl.ap[0], [1, K7], [1, SUB]],
                )
                nc.vector.tensor_mul(s[:, :, :], q_bc, k_wn)
                nc.scalar.activation(s[:, :, :], s[:, :, :], AF.Exp)
                return s

            def sub_mid(k, s):
                """den (PE), ev (DVE), rden (DVE recip), num (PE) for sub k."""
                co, ch = k // NSUB, k % NSUB
                w0 = ch * SUB
                # den = sum_j e_j : 7 accumulating identity matmuls
                den = dpsum.tile([128, SUB], f32, name=f"d{k}", tag="d")
                for j in range(K7):
                    nc.tensor.matmul(
                        den[:, :],
                        idb[:, :],
                        s[:, j, :],
                        start=(j == 0),
                        stop=(j == K7 - 1),
                    )
                # ev = e * v_window (separate tile: PE reads s concurrently)
                ev = epool.tile([128, K7, SUB], bf16, name=f"e{k}", tag="e")
                vsl = vsb[:, co, w0 : w0 + SUB]
                v_w = bass.AP(
                    tensor=vsl.tensor,
                    offset=vsl.offset,
                    ap=[vsl.ap[0], [1, K7], [1, SUB]],
                )
                nc.vector.tensor_mul(ev[:, :, :], s[:, :, :], v_w)
                # rden = 1/den (fast custom-DVE reciprocal, fp32)
                rden = rpool.tile([128, SUB], f32, name=f"r{k}", tag="r")
                nc.vector.reciprocal_approx_fast(out=rden[:, :], in_=den[:, :])
                # num = sum_j ev_j
                num = npsum.tile([128, SUB], f32, name=f"n{k}", tag="n")
                for j in range(K7):
                    nc.tensor.matmul(
                        num[:, :],
                        idb[:, :],
                        ev[:, j, :],
                        start=(j == 0),
                        stop=(j == K7 - 1),
                    )
                return rden, num

            ocur = [None]  # current [128, 2*SUB] out tile (paired stores)

            def sub_final(k, rden, num):
                co, ch = k // NSUB, k % NSUB
                co_sl = slice(co * 128, (co + 1) * 128)
                if ch % 2 == 0:
                    ocur[0] = opool.tile([128, 2 * SUB], bf16, name=f"o{k}", tag="o")
                oc = ocur[0]
                h = ch % 2
                nc.vector.tensor_mul(oc[:, h * SUB : (h + 1) * SUB], num[:, :], rden[:, :])
                if h == 1:
                    w0 = (ch - 1) * SUB
                    nc.sync.dma_start(
                        out=out_d[co_sl, w0 : w0 + 2 * SUB], in_=oc[:, :]
                    )

            NTOT = 2 * NSUB  # 16 flat sub-chunks, co-major
            live = {}  # k -> (s) or (rden, num)
            need_gemms(0, 0)

            # Finish the HAM ramp: 16 more warmup matmuls right after the
            # first gemm group, into a den-pool tile (recycled long before
            # den_1 needs the slot).
            wps2 = dpsum.tile([128, SUB], f32, name="wps2", tag="d")
            for i in range(16):
                nc.tensor.matmul(
                    wps2[:, 0:128],
                    idb[:, :],
                    idb[:, :],
                    start=True,
                    stop=True,
                    skip_group_check=True,
                )
            for r in range(NTOT + 2):
                # prefetch gemm groups for sub r+1's scores window
                if r + 1 < NTOT:
                    co2, ch2 = (r + 1) // NSUB, (r + 1) % NSUB
                    need_gemms(co2, (ch2 + 1) // 2)
                if r < NTOT:
                    live[r] = ("s", sub_scores(r))
                if 1 <= r <= NTOT:
                    k = r - 1
                    live[k] = ("rn", sub_mid(k, live[k][1]))
                if 2 <= r:
                    k = r - 2
                    rden, num = live.pop(k)[1]
                    sub_final(k, rden, num)

    nc.finalize()
    return nc


def _get_nc():
    if "nc" not in _STATE:
        _STATE["nc"] = _build_nc()
    return _STATE["nc"]


def make_in_maps(x, wq, wk, wv):
    import ml_dtypes

    bf = ml_dtypes.bfloat16
    x = np.asarray(x, dtype=np.float32)
    wqT = np.ascontiguousarray(np.asarray(wq, dtype=np.float32).T).astype(bf)
    wkT = np.ascontiguousarray(np.asarray(wk, dtype=np.float32).T).astype(bf)
    wvT = np.ascontiguousarray(np.asarray(wv, dtype=np.float32).T).astype(bf)
    xb = x.astype(bf)
    ident = np.eye(128, dtype=np.float32).astype(bf)
    return [
        {
            "x": np.ascontiguousarray(xb[b]),
            "wtq": wqT,
            "wtk": wkT,
            "wtv": wvT,
            "ident": ident,
        }
        for b in range(B)
    ]


def kernel(x, wq, wk, wv):
    nc = _get_nc()
    in_maps = make_in_maps(x, wq, wk, wv)

    from concourse.bass_utils import run_bass_kernel_spmd

    res = run_bass_kernel_spmd(nc, in_maps, core_ids=list(range(B)))
    outs = [np.asarray(res.results[i]["out"], dtype=np.float32) for i in range(B)]
    return np.stack(outs)


# revision 13
# speedup vs baseline: 1.0047x; 1.0047x over previous
"""AttentionConv kernel for Trainium2 (8 NeuronCores, SPMD data-parallel over batch).

Problem: per-channel windowed softmax attention.
  q = wq @ x; k = wk @ pad(x, 3); v = wv @ pad(x, 3)       (1x1 convs = GEMMs)
  s_j[c,w] = q[c,w] * k[c,w+j],  j = 0..6
  out[c,w] = sum_j softmax_j(s)[c,w,j] * v[c,w+j]

Sharding: batch B=8 -> one batch element per core; weights replicated.
Since pad commutes with the channel-mixing GEMM, k/v are computed on the
unpadded x and written into SBUF buffers with 3 zero columns on each side.

Per-core engine mapping (v2 — window sums moved to the tensor engine):
  TensorE: 3 GEMMs (256x256 @ 256x4096) into fp32 PSUM, plus the softmax
           numerator/denominator j-sums as 7-deep accumulating matmuls with
           an identity stationary (den = sum_j e_j, num = sum_j e_j*v_j),
           which frees ~45% of the vector engine.
  ScalarE: PSUM->SBUF evacuation casts and exp.
  VectorE: windowed score mult, e*v mult, 1/den via the fast custom-DVE
           reciprocal (fp32), final out = num * rden.
The work is software-pipelined over 16 sub-chunks of 512 columns so DVE
(~5.2us), ACT (~4.8us) and PE (~4.4us) overlap per round.
"""

import sys

sys.path.insert(0, "/opt/trn_rl_repo")

import numpy as np

B, C, W = 8, 256, 4096
K7, PAD = 7, 3
SUB = 512  # attention pipeline sub-chunk (den/num PSUM bank)
GRP = 1024  # gemm group columns (one psum tile, 2 banks)
NSUB = W // SUB  # 8 per co block
NGRP = W // GRP  # 4 per co block
WP = W + 2 * PAD

_STATE = {}


def _build_nc():
    import concourse.bass as bass
    import concourse.tile as tile
    from concourse import bacc, mybir

    bf16 = mybir.dt.bfloat16
    f32 = mybir.dt.float32
    AF = mybir.ActivationFunctionType

    nc = bacc.Bacc("TRN2", target_bir_lowering=False, debug=False, num_devices=8)

    x_d = nc.declare_dram_parameter("x", [C, W], bf16, isOutput=False)
    w_d = {
        t: nc.declare_dram_parameter(f"wt{t}", [C, C], bf16, isOutput=False)
        for t in "qkv"
    }
    id_d = nc.declare_dram_parameter("ident", [128, 128], bf16, isOutput=False)
    out_d = nc.declare_dram_parameter("out", [C, W], bf16, isOutput=True)

    with tile.TileContext(nc) as tc:
        from contextlib import ExitStack

        with ExitStack() as ctx:
            persist = ctx.enter_context(tc.tile_pool(name="persist", bufs=1))
            gpsum = ctx.enter_context(tc.tile_pool(name="gpsum", bufs=2, space="PSUM"))
            dpsum = ctx.enter_context(tc.tile_pool(name="dpsum", bufs=2, space="PSUM"))
            npsum = ctx.enter_context(tc.tile_pool(name="npsum", bufs=2, space="PSUM"))
            spool = ctx.enter_context(tc.tile_pool(name="spool", bufs=3))
            epool = ctx.enter_context(tc.tile_pool(name="epool", bufs=3))
            rpool = ctx.enter_context(tc.tile_pool(name="rpool", bufs=3))
            opool = ctx.enter_context(tc.tile_pool(name="opool", bufs=3))

            # ---- persistent SBUF tensors ----
            xb = persist.tile([128, 2, W], bf16, tag="xb")
            wsb = {
                t: persist.tile([128, 2, C], bf16, name=f"wsb_{t}", tag=f"wsb_{t}")
                for t in "qkv"
            }  # w.T, ci-major blocks
            qsb = persist.tile([128, 2, W], bf16, tag="qsb")
            ksb = persist.tile([128, 2, WP], bf16, tag="ksb")
            vsb = persist.tile([128, 2, WP], bf16, tag="vsb")
            idb = persist.tile([128, 128], bf16, tag="idb")
            warm = persist.tile([128, 1], bf16, tag="warm")

            # ---- loads. Critical path (Sync queue): ident -> wq -> x piece 0,
            # which gates the first gemm. The rest (wk/wv, x pieces 1-3) is
            # triggered from the Scalar queue, idle until the first evac, so
            # the Sync queue reaches the x0 trigger ~4us sooner. ----
            for cb in range(2):
                nc.sync.dma_start(
                    out=xb[:, cb, 0:GRP], in_=x_d[cb * 128 : (cb + 1) * 128, 0:GRP]
                )
            nc.sync.dma_start(out=idb[:, :], in_=id_d[:, :])
            for cb in range(2):
                nc.sync.dma_start(
                    out=wsb["q"][:, cb, :], in_=w_d["q"][cb * 128 : (cb + 1) * 128, :]
                )

            # zero the ACT warmup tile before the exp below reads it
            nc.gpsimd.memset(warm[:, :], 0.0)

            # First ACT instruction is a tiny exp so the one table set
            # (exp + copy) loads at t~7us, off the critical path.
            nc.scalar.activation(warm[:, :], warm[:, :], AF.Exp)

            for t in "kv":
                for cb in range(2):
                    nc.scalar.dma_start(
                        out=wsb[t][:, cb, :], in_=w_d[t][cb * 128 : (cb + 1) * 128, :]
                    )
            for p in range(1, NGRP):
                for cb in range(2):
                    nc.scalar.dma_start(
                        out=xb[:, cb, p * GRP : (p + 1) * GRP],
                        in_=x_d[cb * 128 : (cb + 1) * 128, p * GRP : (p + 1) * GRP],
                    )

            # zero the pad columns of k/v (gpsimd keeps these off the busy
            # engines)
            for buf in (ksb, vsb):
                for cb in range(2):
                    nc.gpsimd.memset(buf[:, cb, 0:PAD], 0.0)
                    nc.gpsimd.memset(buf[:, cb, W + PAD : WP], 0.0)

            # PE warmup on the identity tiles: the HAM clock state follows
            # sustained matmul activity, and the steady-state matmul rate for
            # the whole kernel depends on it (measured 216 vs 259 ns/matmul).
            # Split the burst so the first gemm group is not delayed: 8
            # matmuls start the ramp while the x DMA lands; 16 more are
            # emitted right after the first group (see below) to finish it.
            wps = gpsum.tile([128, 128], f32, name="wps", tag="g")

            def pe_warmup(n):
                for i in range(n):
                    nc.tensor.matmul(
                        wps[:, :],
                        idb[:, :],
                        idb[:, :],
                        start=True,
                        stop=True,
                        skip_group_check=True,
                    )

            pe_warmup(16)

            emitted = [0, 0]  # gemm groups emitted per co block

            def gemm_group(co, g):
                """q/k/v GEMM for cols [g*GRP, (g+1)*GRP) of co block."""
                co_sl = slice(co * 128, (co + 1) * 128)
                c0 = g * GRP
                for t in "qkv":
                    ps = gpsum.tile([128, GRP], f32, name=f"g{co}{g}{t}", tag="g")
                    for h in range(GRP // 512):
                        for ci in range(2):
                            nc.tensor.matmul(
                                ps[:, h * 512 : (h + 1) * 512],
                                wsb[t][:, ci, co_sl],
                                xb[:, ci, c0 + h * 512 : c0 + (h + 1) * 512],
                                start=(ci == 0),
                                stop=(ci == 1),
                            )
                    if t == "q":
                        dst = qsb[:, co, c0 : c0 + GRP]
                    else:
                        buf = ksb if t == "k" else vsb
                        dst = buf[:, co, PAD + c0 : PAD + c0 + GRP]
                    nc.scalar.copy(out=dst, in_=ps[:, :])

            def need_gemms(co, upto):
                for g in range(emitted[co], min(upto, NGRP - 1) + 1):
                    gemm_group(co, g)
                emitted[co] = max(emitted[co], min(upto, NGRP - 1) + 1)

            def sub_scores(k):
                """scores + exp for flat sub-chunk k; returns the e tile."""
                co, ch = k // NSUB, k % NSUB
                w0 = ch * SUB
                s = spool.tile([128, K7, SUB], bf16, name=f"s{k}", tag="s")
                qsl = qsb[:, co, w0 : w0 + SUB]
                ksl = ksb[:, co, w0 : w0 + SUB]
                q_bc = bass.AP(
                    tensor=qsl.tensor,
                    offset=qsl.offset,
                    ap=[qsl.ap[0], [0, K7], [1, SUB]],
                )
                k_wn = bass.AP(
                    tensor=ksl.tensor,
                    offset=ksl.offset,
                    ap=[ksl.ap[0], [1, K7], [1, SUB]],
                )
                nc.vector.tensor_mul(s[:, :, :], q_bc, k_wn)
                nc.scalar.activation(s[:, :, :], s[:, :, :], AF.Exp)
                return s

            def sub_mid(k, s):
                """den (PE), ev (DVE), rden (DVE recip), num (PE) for sub k."""
                co, ch = k // NSUB, k % NSUB
                w0 = ch * SUB
                # den = sum_j e_j : 7 accumulating identity matmuls
                den = dpsum.tile([128, SUB], f32, name=f"d{k}", tag="d")
                for j in range(K7):
                    nc.tensor.matmul(
                        den[:, :],
                        idb[:, :],
                        s[:, j, :],
                        start=(j == 0),
                        stop=(j == K7 - 1),
                    )
                # ev = e * v_window (separate tile: PE reads s concurrently)
                ev = epool.tile([128, K7, SUB], bf16, name=f"e{k}", tag="e")
                vsl = vsb[:, co, w0 : w0 + SUB]
                v_w = bass.AP(
                    tensor=vsl.tensor,
                    offset=vsl.offset,
                    ap=[vsl.ap[0], [1, K7], [1, SUB]],
                )
                nc.vector.tensor_mul(ev[:, :, :], s[:, :, :], v_w)
                # rden = 1/den (fast custom-DVE reciprocal, fp32)
                rden = rpool.tile([128, SUB], f32, name=f"r{k}", tag="r")
                nc.vector.reciprocal_approx_fast(out=rden[:, :], in_=den[:, :])
                # num = sum_j ev_j
                num = npsum.tile([128, SUB], f32, name=f"n{k}", tag="n")
                for j in range(K7):
                    nc.tensor.matmul(
                        num[:, :],
                        idb[:, :],
                        ev[:, j, :],
                        start=(j == 0),
                        stop=(j == K7 - 1),
                    )
                return rden, num

            ocur = [None]  # current [128, 2*SUB] out tile (paired stores)

            def sub_final(k, rden, num):
                co, ch = k // NSUB, k % NSUB
                co_sl = slice(co * 128, (co + 1) * 128)
                if ch % 2 == 0:
                    ocur[0] = opool.tile([128, 2 * SUB], bf16, name=f"o{k}", tag="o")
                oc = ocur[0]
                h = ch % 2
                nc.vector.tensor_mul(oc[:, h * SUB : (h + 1) * SUB], num[:, :], rden[:, :])
                if h == 1:
                    w0 = (ch - 1) * SUB
                    nc.sync.dma_start(
                        out=out_d[co_sl, w0 : w0 + 2 * SUB], in_=oc[:, :]
                    )

            NTOT = 2 * NSUB  # 16 flat sub-chunks, co-major
            live = {}  # k -> (s) or (rden, num)
            need_gemms(0, 0)

            # Finish the HAM ramp: 16 more warmup matmuls right after the
            # first gemm group, into a den-pool tile (recycled long before
            # den_1 needs the slot).
            wps2 = dpsum.tile([128, SUB], f32, name="wps2", tag="d")
            for i in range(16):
                nc.tensor.matmul(
                    wps2[:, 0:128],
                    idb[:, :],
                    idb[:, :],
                    start=True,
                    stop=True,
                    skip_group_check=True,
                )
            for r in range(NTOT + 2):
                # prefetch gemm groups for sub r+1's scores window
                if r + 1 < NTOT:
                    co2, ch2 = (r + 1) // NSUB, (r + 1) % NSUB
                    need_gemms(co2, (ch2 + 1) // 2)
                if r < NTOT:
                    live[r] = ("s", sub_scores(r))
                if 1 <= r <= NTOT:
                    k = r - 1
                    live[k] = ("rn", sub_mid(k, live[k][1]))
                if 2 <= r:
                    k = r - 2
                    rden, num = live.pop(k)[1]
                    sub_final(k, rden, num)

    nc.finalize()
    return nc


def _get_nc():
    if "nc" not in _STATE:
        _STATE["nc"] = _build_nc()
    return _STATE["nc"]


def make_in_maps(x, wq, wk, wv):
    import ml_dtypes

    bf = ml_dtypes.bfloat16
    x = np.asarray(x, dtype=np.float32)
    wqT = np.ascontiguousarray(np.asarray(wq, dtype=np.float32).T).astype(bf)
    wkT = np.ascontiguousarray(np.asarray(wk, dtype=np.float32).T).astype(bf)
    wvT = np.ascontiguousarray(np.asarray(wv, dtype=np.float32).T).astype(bf)
    xb = x.astype(bf)
    ident = np.eye(128, dtype=np.float32).astype(bf)
    return [
        {
            "x": np.ascontiguousarray(xb[b]),
            "wtq": wqT,
            "wtk": wkT,
            "wtv": wvT,
            "ident": ident,
        }
        for b in range(B)
    ]


def kernel(x, wq, wk, wv):
    nc = _get_nc()
    in_maps = make_in_maps(x, wq, wk, wv)

    from concourse.bass_utils import run_bass_kernel_spmd

    res = run_bass_kernel_spmd(nc, in_maps, core_ids=list(range(B)))
    outs = [np.asarray(res.results[i]["out"], dtype=np.float32) for i in range(B)]
    return np.stack(outs)
